# revision 3
# baseline (speedup 1.0000x reference)
"""Trainium2 Bass kernel for nn_MultiHeadAttention_55336358642102 (v3).

Data-parallel over the 8 sentences (one per core, no collectives).
All heavy matmuls are fp8-e4m3 DoubleRow (2 contraction rows/cycle).

v3 over v2:
  - S-score psum is a 3-slot rotation (4+2+2 key-chunks -> exp FD
    2048/1024/1024) so the scalar-engine exp stream never WAR-stalls
    against the next S matmuls.
  - softmax denominator and PV share one psum bank (dd -> recip -> pv
    reuses the slot), freeing a bank for the rotation.
  - O normalization is a two-hop drain: DVE copies psum->bf16 at 2x,
    GPSIMD multiplies by the replicated 1/D and writes fp8 into the
    packed projection operands.  Q/K drains likewise (except pair 0,
    which stays on DVE to shorten the critical path to the first exp).
  - PE warmup matmuls raise the tensor-engine p-state before the first
    real work; input DMAs are issued in dependency order.
  - layernorm without any activation-table switch: Sum(z) and Sum(z^2)
    come from scalar-engine Identity/Square accumulators (both live in
    the exp table set), 1/(sigma+eps) from a batched Newton rsqrt on DVE
    (quake-style integer seed + 2 iterations), then one tensor_scalar.
"""

import sys

import ml_dtypes
import numpy as np

if "/opt/trn_rl_repo" not in sys.path:
    sys.path.insert(0, "/opt/trn_rl_repo")

import concourse.bass as bass
import concourse.mybir as mybir
import concourse.tile as tile
from concourse import bacc
from concourse.bass import ds
from concourse.bass_utils import run_bass_kernel_spmd

P = 128
L = 1024
DM = 1024
NCORES = 8
EPS = 1e-3
WS = 64.0
QS = 16.0
RES = 4096.0
EXP_SCALE = 1.0 / (QS * QS * 32.0)
N = 1024.0                      # layernorm feature count
MAGIC = 0x5F3759DF

F32 = mybir.dt.float32
BF16 = mybir.dt.bfloat16
F8 = mybir.dt.float8e4
I32 = mybir.dt.int32
AF = mybir.ActivationFunctionType
ALU = mybir.AluOpType
PM = mybir.MatmulPerfMode.DoubleRow
F8NP = ml_dtypes.float8_e4m3
BF16NP = ml_dtypes.bfloat16


def build_nc(apply_ln: bool) -> bass.Bass:
    nc = bacc.Bacc(None, target_bir_lowering=False)

    xt_d = nc.dram_tensor("xt", [P, 8, L], F8, kind="ExternalInput")
    wqk_d = nc.dram_tensor("wqk", [P, 4, 2, 8, P], F8, kind="ExternalInput")
    wv_d = nc.dram_tensor("wv", [P, 8, 512], F8, kind="ExternalInput")
    w1_d = nc.dram_tensor("w1", [P, 4, 768], F8, kind="ExternalInput")
    w2_d = nc.dram_tensor("w2", [P, 4, 256], F8, kind="ExternalInput")
    id_d = nc.dram_tensor("ident", [P, P], BF16, kind="ExternalInput")
    xn_d = nc.dram_tensor("xn", [P, 8, DM], BF16, kind="ExternalInput")
    if apply_ln:
        lna_d = nc.dram_tensor("lna", [P, DM], F32, kind="ExternalInput")
        lnb_d = nc.dram_tensor("lnb", [P, DM], F32, kind="ExternalInput")
    out_d = nc.dram_tensor("out", [L, DM], F32, kind="ExternalOutput")

    with tile.TileContext(nc) as tc:
        with (
            tc.tile_pool(name="sing", bufs=1) as sing,
            tc.tile_pool(name="qkp", bufs=3) as qkp,
            tc.tile_pool(name="stg", bufs=4) as stg,
            tc.tile_pool(name="ep", bufs=8) as ep,
            tc.tile_pool(name="rdp", bufs=6) as rdp,
            tc.tile_pool(name="outp", bufs=2) as outp,
            tc.tile_pool(name="statp", bufs=4) as statp,
            tc.tile_pool(name="ps", bufs=1, space="PSUM") as ps,
        ):
            # resident inputs, in dependency order
            XT = sing.tile([P, 8, L], F8)
            nc.sync.dma_start(XT, xt_d[:])
            WQK = sing.tile([P, 4, 2, 8, P], F8)
            nc.sync.dma_start(WQK, wqk_d[:])
            WV = sing.tile([P, 8, 512], F8)
            nc.sync.dma_start(WV, wv_d[:])
            W1 = sing.tile([P, 4, 768], F8)
            nc.sync.dma_start(W1, w1_d[:])
            W2 = sing.tile([P, 4, 256], F8)
            nc.sync.dma_start(W2, w2_d[:])
            ID = sing.tile([P, P], BF16)
            nc.sync.dma_start(ID, id_d[:])
            XN = sing.tile([P, 8, DM], BF16)
            nc.sync.dma_start(XN, xn_d[:])
            if apply_ln:
                LNA = sing.tile([P, DM], F32)
                nc.sync.dma_start(LNA, lna_d[:])
                LNB = sing.tile([P, DM], F32)
                nc.sync.dma_start(LNB, lnb_d[:])

            ones8 = sing.tile([P, 2, P], F8)
            nc.vector.memset(ones8, 1.0)

            V = sing.tile([P, 8, 1024], F8)
            O1T = sing.tile([P, 4, L], F8)
            O2T = sing.tile([P, 4, L], F8)

            # PE p-state warmup: garbage DR matmuls with no DMA deps
            warm = sing.tile([P, 2, 512], F8)
            nc.vector.memset(warm, 1.0)
            for w in range(24):
                pw = ps.tile([P, 512], F32, tag="pvdd", name=f"warm{w}")
                nc.tensor.matmul(pw, ones8, warm, start=True, stop=True,
                                 perf_mode=PM)

            QT = [qkp.tile([P, 2, L], F8, tag="qt", name=f"qt{j}") for j in range(4)]
            KT = [qkp.tile([P, 2, L], F8, tag="kt", name=f"kt{j}") for j in range(4)]

            def qk_round(j, s, half, direct):
                """Q/K production for (pair, q-or-k, token-half).

                Two sequential [128,512] psums (content then positional)
                through the 1-bank "qk" slot; drain either directly to fp8
                on DVE (direct=True) or via bf16 staging + gpsimd cast.
                """
                hs = ds(half * 512, 512)
                dst = (QT[j] if s == 0 else KT[j])
                for grp in range(2):
                    pq = ps.tile([P, 512], F32, tag="qk", name=f"q{j}{s}{half}{grp}")
                    if grp == 0:
                        for kk in range(3):
                            nc.tensor.matmul(pq, WQK[:, j, s, 2 * kk:2 * kk + 2, :],
                                             XT[:, 2 * kk:2 * kk + 2, hs],
                                             start=(kk == 0), stop=(kk == 2),
                                             perf_mode=PM)
                    else:
                        nc.tensor.matmul(pq, WQK[:, j, s, 6:8, :], XT[:, 6:8, hs],
                                         start=True, stop=True, perf_mode=PM)
                    if direct:
                        nc.vector.tensor_scalar(dst[:, grp, hs], pq,
                                                1.0 / 4.0, None, ALU.mult)
                    else:
                        sg = stg.tile([P, 512], BF16, tag="qs",
                                      name=f"qs{j}{s}{half}{grp}")
                        nc.vector.tensor_scalar(sg, pq, 1.0 / 4.0, None, ALU.mult)
                        nc.gpsimd.tensor_copy(dst[:, grp, hs], sg)

            def v_step(rc):
                """V production for one 128-key chunk."""
                rsl = ds(rc * P, P)
                for grp in range(2):
                    pv_ = ps.tile([P, 512], F32, tag="qk", name=f"v{rc}{grp}")
                    if grp == 0:
                        for kk in range(3):
                            nc.tensor.matmul(pv_, XT[:, 2 * kk:2 * kk + 2, rsl],
                                             WV[:, 2 * kk:2 * kk + 2, :],
                                             start=(kk == 0), stop=(kk == 2),
                                             perf_mode=PM)
                    else:
                        nc.tensor.matmul(pv_, XT[:, 6:8, rsl], WV[:, 6:8, :],
                                         start=True, stop=True, perf_mode=PM)
                    vv = V[:, rc].rearrange("p (j x) -> p j x", j=4)
                    o = 0 if grp == 0 else 64      # cont -> slot base 0/192, pos -> 64/128
                    s1 = vv[:, :, o:o + 64]
                    s2 = vv[:, :, 192 - o:256 - o]
                    nc.vector.tensor_copy(s1, pv_[:, 0:256].rearrange("p (j e) -> p j e", j=4))
                    nc.vector.tensor_copy(s2, pv_[:, 256:512].rearrange("p (j e) -> p j e", j=4))

            # pair-0 Q/K on the fast (direct) drain path
            for s in range(2):
                for half in range(2):
                    qk_round(0, s, half, direct=True)

            prod_sched = ([("qk", 1, s, half) for s in range(2) for half in range(2)] +
                          [("v", rc) for rc in range(8)] +
                          [("qk", j, s, half) for j in (2, 3)
                           for s in range(2) for half in range(2)])
            # 20 production steps; pair1 by i=2, V by i=6, pair2 by i=8
            prod_per_iter = [2, 2, 2, 2, 2, 2, 2, 2, 2, 2, 0, 0, 0, 0, 0, 0]

            def emit_prod(i):
                for _ in range(prod_per_iter[i]):
                    if prod_sched:
                        stp = prod_sched.pop(0)
                        if stp[0] == "v":
                            v_step(stp[1])
                        else:
                            qk_round(stp[1], stp[2], stp[3], direct=False)

            LAG = 6
            hist = {}

            def attn_front(i):
                j, hh, half = i // 4, (i // 2) % 2, i % 2
                hs = ds(half * 512, 512)
                pb = ds(64 * hh, 64)
                eA = ep.tile([P, 4, 512], F8, tag="e", name=f"eA{i}")
                eB = ep.tile([P, 2, 512], F8, tag="eb", name=f"eB{i}")
                eC = ep.tile([P, 2, 512], F8, tag="ec", name=f"eC{i}")
                sA = ps.tile([P, 2048], F32, tag="sA")
                for c in range(4):
                    nc.tensor.matmul(sA[:, ds(512 * c, 512)],
                                     KT[j][pb, :, ds(128 * c, 128)],
                                     QT[j][pb, :, hs], start=True, stop=True,
                                     perf_mode=PM)
                nc.scalar.activation(eA.rearrange("p a b -> p (a b)"), sA,
                                     AF.Exp, scale=EXP_SCALE)
                sB = ps.tile([P, 1024], F32, tag="sB", name=f"sB{i}")
                for c in range(2):
                    nc.tensor.matmul(sB[:, ds(512 * c, 512)],
                                     KT[j][pb, :, ds(128 * (4 + c), 128)],
                                     QT[j][pb, :, hs], start=True, stop=True,
                                     perf_mode=PM)
                nc.scalar.activation(eB.rearrange("p a b -> p (a b)"), sB,
                                     AF.Exp, scale=EXP_SCALE)
                if i >= LAG:
                    attn_back(i - LAG, *hist.pop(i - LAG))
                sC = ps.tile([P, 1024], F32, tag="sB", name=f"sC{i}")
                for c in range(2):
                    nc.tensor.matmul(sC[:, ds(512 * c, 512)],
                                     KT[j][pb, :, ds(128 * (6 + c), 128)],
                                     QT[j][pb, :, hs], start=True, stop=True,
                                     perf_mode=PM)
                nc.scalar.activation(eC.rearrange("p a b -> p (a b)"), sC,
                                     AF.Exp, scale=EXP_SCALE)
                emit_prod(i)
                hist[i] = (eA, eB, eC)

            def attn_back(i, eA, eB, eC):
                j, hh, half = i // 4, (i // 2) % 2, i % 2
                hs = ds(half * 512, 512)
                h = 2 * j + hh
                dd = ps.tile([P, 512], F32, tag="pvdd", name=f"dd{i}")
                for cc in range(4):
                    e = (eA[:, 0:2], eA[:, 2:4], eB, eC)[cc]
                    nc.tensor.matmul(dd, ones8, e,
                                     start=(cc == 0), stop=(cc == 3), perf_mode=PM)
                rd = rdp.tile([P, 512], F32, tag="rd", name=f"rd{i}")
                nc.vector.reciprocal_approx_fast(rd, dd)
                pv = ps.tile([P, 512], F32, tag="pvdd", name=f"pv{i}")
                for cc in range(4):
                    e = (eA[:, 0:2], eA[:, 2:4], eB, eC)[cc]
                    nc.tensor.matmul(pv, V[:, 2 * cc:2 * cc + 2, ds(128 * h, 128)],
                                     e, start=(cc == 0), stop=(cc == 3), perf_mode=PM)
                ob = stg.tile([P, 512], BF16, tag="ob", name=f"ob{i}")
                nc.vector.tensor_copy(ob, pv)
                if hh == 0:
                    nc.gpsimd.tensor_mul(O1T[0:64, j, hs], ob[0:64], rd[0:64])
                    nc.gpsimd.tensor_mul(O2T[64:128, j, hs], ob[64:128], rd[64:128])
                else:
                    nc.gpsimd.tensor_mul(O2T[0:64, j, hs], ob[0:64], rd[0:64])
                    nc.gpsimd.tensor_mul(O1T[64:128, j, hs], ob[64:128], rd[64:128])

            for i in range(16):
                attn_front(i)
            for i in range(16 - LAG, 16):
                attn_back(i, *hist.pop(i))

            # ---- phase C ------------------------------------------------
            CT = sing.tile([P, 8, 2], F32)       # per-block [sum(z), sum(z^2)]
            MF = sing.tile([P, 8, 2], F32)       # per-block [mu, factor]
            zs = {}

            def proj_block(tb, zt, zoff):
                tsl = ds(tb * P, P)
                z = zt[:, ds(zoff, 1024)]
                nc.tensor.matmul(z[:, 0:512], ID, XN[:, tb, 0:512],
                                 start=True, stop=True)
                nc.tensor.matmul(z[:, 512:1024], ID, XN[:, tb, 512:1024],
                                 start=True, stop=True)
                for jj in range(2):
                    nc.tensor.matmul(z[:, 0:512], O1T[:, 2 * jj:2 * jj + 2, tsl],
                                     W1[:, 2 * jj:2 * jj + 2, 0:512],
                                     start=False, stop=(jj == 1), perf_mode=PM,
                                     skip_group_check=True)
                for jj in range(2):
                    nc.tensor.matmul(z[:, 512:768], O1T[:, 2 * jj:2 * jj + 2, tsl],
                                     W1[:, 2 * jj:2 * jj + 2, 512:768],
                                     start=False, stop=(jj == 1), perf_mode=PM,
                                     skip_group_check=True)
                for jj in range(2):
                    nc.tensor.matmul(z[:, 768:1024], O2T[:, 2 * jj:2 * jj + 2, tsl],
                                     W2[:, 2 * jj:2 * jj + 2, :],
                                     start=False, stop=(jj == 1), perf_mode=PM,
                                     skip_group_check=True)
                dmy = statp.tile([P, 1024], BF16, tag="dmy", name=f"dm{tb}")
                nc.scalar.activation(dmy, z, AF.Identity,
                                     accum_out=CT[:, tb, 0:1])
                nc.scalar.activation(dmy, z, AF.Square,
                                     accum_out=CT[:, tb, 1:2])
                zs[tb] = z

            def newton(t0, b=1):
                """1/(sigma+eps) for blocks [t0, t0+b) from CT -> MF."""
                sz = CT[:, ds(t0, b), 0]
                sq = CT[:, ds(t0, b), 1]
                mu = MF[:, ds(t0, b), 0]
                fc = MF[:, ds(t0, b), 1]
                w = statp.tile([P, b], F32, tag="w", name=f"w{t0}")
                y = statp.tile([P, b], F32, tag="y", name=f"y{t0}")
                t = statp.tile([P, b], F32, tag="t", name=f"t{t0}")
                nc.vector.tensor_scalar(mu, sz, 1.0 / N, None, ALU.mult)
                nc.vector.tensor_mul(w, sz, sz)
                nc.vector.tensor_scalar(w, w, -1.0 / N, None, ALU.mult)
                nc.vector.tensor_add(w, w, sq)      # w = (n-1) * unbiased var
                wi = w.bitcast(I32)
                yi = y.bitcast(I32)
                nc.vector.tensor_scalar(yi, wi, 1, None,
                                        ALU.logical_shift_right)
                nc.vector.tensor_scalar(yi, yi, -1, None, ALU.mult)
                nc.vector.tensor_scalar(yi, yi, MAGIC, None, ALU.add)
                for _ in range(2):                  # Newton: y = y*(1.5-0.5*w*y^2)
                    nc.vector.tensor_mul(t, w, y)
                    nc.vector.tensor_mul(t, t, y)
                    nc.vector.tensor_scalar(t, t, -0.5, 1.5, ALU.mult, ALU.add)
                    nc.vector.tensor_mul(y, y, t)
                # factor = sqrt(n-1)*y/(1+eps'*sqrt(n-1)*y) ~= y*(c0 - c1*y)
                c0 = float(np.sqrt(N - 1.0))
                c1 = float(RES * EPS * (N - 1.0))
                nc.vector.tensor_scalar(t, y, -c1, c0, ALU.mult, ALU.add)
                nc.vector.tensor_mul(fc, y, t)

            def norm_block(tb):
                tsl = ds(tb * P, P)
                ot = outp.tile([P, DM], F32, tag="o", name=f"ot{tb}")
                nc.vector.tensor_scalar(ot, zs[tb], MF[:, tb, 0:1],
                                        MF[:, tb, 1:2], ALU.subtract, ALU.mult)
                if apply_ln:
                    nc.vector.tensor_mul(ot, ot, LNA)
                    nc.vector.tensor_add(ot, ot, LNB)
                nc.sync.dma_start(out_d[tsl, :], ot)

            # z slots: sA holds two blocks (halves), sB one; per-block newton
            # so blocks pipeline across PE -> ACT -> DVE.
            slot_of = [0, 1, 2, 0, 1, 2, 0, 1]   # 0/1 = sA half, 2 = sB
            zA = None
            for tb in range(8):
                sl = slot_of[tb]
                if sl == 2:
                    zt = ps.tile([P, 1024], F32, tag="sB", name=f"zB{tb}")
                    proj_block(tb, zt, 0)
                else:
                    if sl == 0:
                        zA = ps.tile([P, 2048], F32, tag="sA", name=f"zA{tb}")
                    proj_block(tb, zA, 1024 * sl)
                newton(tb)
                norm_block(tb)

    nc.finalize()
    return nc


def _prep(inp, w_qs1, w_ks1, w_vs1, w_qs2, w_ks2, w_vs2, w_proj1, w_proj2):
    x = np.ascontiguousarray(np.asarray(inp, np.float32)).reshape(NCORES, L, DM)

    xts, xns = [], []
    for b in range(NCORES):
        xt = x[b].T.reshape(8, P, L).transpose(1, 0, 2)
        xts.append(np.ascontiguousarray(xt).astype(F8NP))
        xn = x[b].reshape(8, P, DM).transpose(1, 0, 2)
        xns.append(np.ascontiguousarray(xn).astype(BF16NP))

    wqk = np.empty((P, 4, 2, 8, P), np.float32)
    for j in range(4):
        for s, (wa, wb) in enumerate(((w_qs1, w_qs2), (w_ks1, w_ks2))):
            for f in range(6):
                for m_h, h in ((0, 2 * j), (1, 2 * j + 1)):
                    wqk[:, j, s, f, 64 * m_h:64 * m_h + 64] = \
                        wa[h, 128 * f:128 * (f + 1), :]
            for f in (6, 7):
                for m_h, h in ((0, 2 * j), (1, 2 * j + 1)):
                    wqk[:, j, s, f, 64 * m_h:64 * m_h + 64] = \
                        wb[h, 128 * (f - 6):128 * (f - 5), :]
    wqk = (wqk * WS).astype(F8NP)

    horder = [0, 2, 4, 6, 1, 3, 5, 7]
    wv = np.empty((P, 8, 512), np.float32)
    for f in range(6):
        for i, h in enumerate(horder):
            wv[:, f, 64 * i:64 * i + 64] = w_vs1[h, 128 * f:128 * (f + 1), :]
    for f in (6, 7):
        for i, h in enumerate(horder):
            wv[:, f, 64 * i:64 * i + 64] = w_vs2[h, 128 * (f - 6):128 * (f - 5), :]
    wv = (wv * WS).astype(F8NP)

    w1 = np.ascontiguousarray(
        (np.asarray(w_proj1, np.float32) * WS).reshape(4, P, 768)
        .transpose(1, 0, 2)).astype(F8NP)
    w2r = np.asarray(w_proj2, np.float32).reshape(4, 2, 64, 256)
    w2r = np.ascontiguousarray(w2r[:, ::-1].reshape(4, P, 256) * WS)
    w2 = np.ascontiguousarray(w2r.transpose(1, 0, 2)).astype(F8NP)

    ident = (np.eye(P, dtype=np.float32) * RES).astype(BF16NP)
    return x, xts, xns, wqk, wv, w1, w2, ident


_NC_CACHE = {}


def _get_nc(apply_ln):
    if apply_ln not in _NC_CACHE:
        _NC_CACHE[apply_ln] = build_nc(apply_ln)
    return _NC_CACHE[apply_ln]


def kernel(inp, w_qs1, w_ks1, w_vs1, w_qs2, w_ks2, w_vs2, w_proj1, w_proj2,
           ln_a, ln_b, batch_size, max_len, _trace=False):
    inp = np.asarray(inp, np.float32)
    assert int(batch_size) == NCORES and int(max_len) == L
    assert inp.shape == (NCORES * L, DM)

    ln_a = np.asarray(ln_a, np.float32).reshape(-1)
    ln_b = np.asarray(ln_b, np.float32).reshape(-1)
    apply_ln = not (np.all(ln_a == 1.0) and np.all(ln_b == 0.0))

    x, xts, xns, wqk, wv, w1, w2, ident = _prep(
        inp, np.asarray(w_qs1, np.float32), np.asarray(w_ks1, np.float32),
        np.asarray(w_vs1, np.float32), np.asarray(w_qs2, np.float32),
        np.asarray(w_ks2, np.float32), np.asarray(w_vs2, np.float32),
        np.asarray(w_proj1, np.float32), np.asarray(w_proj2, np.float32))

    nc = _get_nc(apply_ln)

    in_maps = []
    for b in range(NCORES):
        m = dict(xt=xts[b], xn=xns[b], wqk=wqk, wv=wv, w1=w1, w2=w2,
                 ident=ident)
        if apply_ln:
            m["lna"] = np.broadcast_to(ln_a, (P, DM)).copy()
            m["lnb"] = np.broadcast_to(ln_b, (P, DM)).copy()
        in_maps.append(m)

    res = run_bass_kernel_spmd(nc, in_maps, list(range(NCORES)), trace=_trace)
    out = np.concatenate([res.results[b]["out"] for b in range(NCORES)], 0)
    if _trace:
        return out, res
    return out


# revision 4
# speedup vs baseline: 1.0431x; 1.0431x over previous
"""Trainium2 Bass kernel for nn_MultiHeadAttention_55336358642102 (v3).

Data-parallel over the 8 sentences (one per core, no collectives).
All heavy matmuls are fp8-e4m3 DoubleRow (2 contraction rows/cycle).

v3 over v2:
  - S-score psum is a 3-slot rotation (4+2+2 key-chunks -> exp FD
    2048/1024/1024) so the scalar-engine exp stream never WAR-stalls
    against the next S matmuls.
  - softmax denominator and PV share one psum bank (dd -> recip -> pv
    reuses the slot), freeing a bank for the rotation.
  - O normalization is a two-hop drain: DVE copies psum->bf16 at 2x,
    GPSIMD multiplies by the replicated 1/D and writes fp8 into the
    packed projection operands.  Q/K drains likewise (except pair 0,
    which stays on DVE to shorten the critical path to the first exp).
  - PE warmup matmuls raise the tensor-engine p-state before the first
    real work; input DMAs are issued in dependency order.
  - layernorm without any activation-table switch: Sum(z) and Sum(z^2)
    come from scalar-engine Identity/Square accumulators (both live in
    the exp table set), 1/(sigma+eps) from a batched Newton rsqrt on DVE
    (quake-style integer seed + 2 iterations), then one tensor_scalar.
"""

import sys

import ml_dtypes
import numpy as np

if "/opt/trn_rl_repo" not in sys.path:
    sys.path.insert(0, "/opt/trn_rl_repo")

import concourse.bass as bass
import concourse.mybir as mybir
import concourse.tile as tile
from concourse import bacc
from concourse.bass import ds
from concourse.bass_utils import run_bass_kernel_spmd

P = 128
L = 1024
DM = 1024
NCORES = 8
EPS = 1e-3
WS = 64.0
QS = 16.0
RES = 4096.0
EXP_SCALE = 1.0 / (QS * QS * 32.0)
N = 1024.0                      # layernorm feature count
MAGIC = 0x5F3759DF

F32 = mybir.dt.float32
BF16 = mybir.dt.bfloat16
F8 = mybir.dt.float8e4
I32 = mybir.dt.int32
AF = mybir.ActivationFunctionType
ALU = mybir.AluOpType
PM = mybir.MatmulPerfMode.DoubleRow
F8NP = ml_dtypes.float8_e4m3
BF16NP = ml_dtypes.bfloat16


def build_nc(apply_ln: bool) -> bass.Bass:
    nc = bacc.Bacc(None, target_bir_lowering=False)

    xt_d = nc.dram_tensor("xt", [P, 8, L], F8, kind="ExternalInput")
    wqk_d = nc.dram_tensor("wqk", [P, 4, 2, 8, P], F8, kind="ExternalInput")
    wv_d = nc.dram_tensor("wv", [P, 8, 512], F8, kind="ExternalInput")
    w1_d = nc.dram_tensor("w1", [P, 4, 768], F8, kind="ExternalInput")
    w2_d = nc.dram_tensor("w2", [P, 4, 256], F8, kind="ExternalInput")
    id_d = nc.dram_tensor("ident", [P, P], BF16, kind="ExternalInput")
    xn_d = nc.dram_tensor("xn", [P, 8, DM], BF16, kind="ExternalInput")
    if apply_ln:
        lna_d = nc.dram_tensor("lna", [P, DM], F32, kind="ExternalInput")
        lnb_d = nc.dram_tensor("lnb", [P, DM], F32, kind="ExternalInput")
    out_d = nc.dram_tensor("out", [L, DM], F32, kind="ExternalOutput")

    with tile.TileContext(nc) as tc:
        with (
            tc.tile_pool(name="sing", bufs=1) as sing,
            tc.tile_pool(name="qkp", bufs=3) as qkp,
            tc.tile_pool(name="stg", bufs=4) as stg,
            tc.tile_pool(name="ep", bufs=8) as ep,
            tc.tile_pool(name="rdp", bufs=6) as rdp,
            tc.tile_pool(name="outp", bufs=2) as outp,
            tc.tile_pool(name="statp", bufs=4) as statp,
            tc.tile_pool(name="ps", bufs=1, space="PSUM") as ps,
        ):
            # resident inputs, in dependency order
            XT = sing.tile([P, 8, L], F8)
            nc.sync.dma_start(XT, xt_d[:])
            WQK = sing.tile([P, 4, 2, 8, P], F8)
            nc.sync.dma_start(WQK, wqk_d[:])
            WV = sing.tile([P, 8, 512], F8)
            nc.sync.dma_start(WV, wv_d[:])
            W1 = sing.tile([P, 4, 768], F8)
            nc.sync.dma_start(W1, w1_d[:])
            W2 = sing.tile([P, 4, 256], F8)
            nc.sync.dma_start(W2, w2_d[:])
            ID = sing.tile([P, P], BF16)
            nc.sync.dma_start(ID, id_d[:])
            XN = sing.tile([P, 8, DM], BF16)
            nc.sync.dma_start(XN, xn_d[:])
            if apply_ln:
                LNA = sing.tile([P, DM], F32)
                nc.sync.dma_start(LNA, lna_d[:])
                LNB = sing.tile([P, DM], F32)
                nc.sync.dma_start(LNB, lnb_d[:])

            ones8 = sing.tile([P, 2, P], F8)
            nc.vector.memset(ones8, 1.0)

            V = sing.tile([P, 8, 1024], F8)
            O1T = sing.tile([P, 4, L], F8)
            O2T = sing.tile([P, 4, L], F8)

            # PE p-state warmup: garbage DR matmuls with no DMA deps
            warm = sing.tile([P, 2, 512], F8)
            nc.vector.memset(warm, 1.0)
            for w in range(24):
                pw = ps.tile([P, 512], F32, tag="pvdd", name=f"warm{w}")
                nc.tensor.matmul(pw, ones8, warm, start=True, stop=True,
                                 perf_mode=PM)

            QT = [qkp.tile([P, 2, L], F8, tag="qt", name=f"qt{j}") for j in range(4)]
            KT = [qkp.tile([P, 2, L], F8, tag="kt", name=f"kt{j}") for j in range(4)]

            def qk_round(j, s, half, direct):
                """Q/K production for (pair, q-or-k, token-half).

                Two sequential [128,512] psums (content then positional)
                through the 1-bank "qk" slot; drain either directly to fp8
                on DVE (direct=True) or via bf16 staging + gpsimd cast.
                """
                hs = ds(half * 512, 512)
                dst = (QT[j] if s == 0 else KT[j])
                for grp in range(2):
                    pq = ps.tile([P, 512], F32, tag="qk", name=f"q{j}{s}{half}{grp}")
                    if grp == 0:
                        for kk in range(3):
                            nc.tensor.matmul(pq, WQK[:, j, s, 2 * kk:2 * kk + 2, :],
                                             XT[:, 2 * kk:2 * kk + 2, hs],
                                             start=(kk == 0), stop=(kk == 2),
                                             perf_mode=PM)
                    else:
                        nc.tensor.matmul(pq, WQK[:, j, s, 6:8, :], XT[:, 6:8, hs],
                                         start=True, stop=True, perf_mode=PM)
                    if direct:
                        nc.vector.tensor_scalar(dst[:, grp, hs], pq,
                                                1.0 / 4.0, None, ALU.mult)
                    else:
                        sg = stg.tile([P, 512], BF16, tag="qs",
                                      name=f"qs{j}{s}{half}{grp}")
                        nc.vector.tensor_scalar(sg, pq, 1.0 / 4.0, None, ALU.mult)
                        nc.gpsimd.tensor_copy(dst[:, grp, hs], sg)

            def v_step(rc):
                """V production for one 128-key chunk."""
                rsl = ds(rc * P, P)
                for grp in range(2):
                    pv_ = ps.tile([P, 512], F32, tag="qk", name=f"v{rc}{grp}")
                    if grp == 0:
                        for kk in range(3):
                            nc.tensor.matmul(pv_, XT[:, 2 * kk:2 * kk + 2, rsl],
                                             WV[:, 2 * kk:2 * kk + 2, :],
                                             start=(kk == 0), stop=(kk == 2),
                                             perf_mode=PM)
                    else:
                        nc.tensor.matmul(pv_, XT[:, 6:8, rsl], WV[:, 6:8, :],
                                         start=True, stop=True, perf_mode=PM)
                    vv = V[:, rc].rearrange("p (j x) -> p j x", j=4)
                    o = 0 if grp == 0 else 64      # cont -> slot base 0/192, pos -> 64/128
                    s1 = vv[:, :, o:o + 64]
                    s2 = vv[:, :, 192 - o:256 - o]
                    nc.vector.tensor_copy(s1, pv_[:, 0:256].rearrange("p (j e) -> p j e", j=4))
                    nc.vector.tensor_copy(s2, pv_[:, 256:512].rearrange("p (j e) -> p j e", j=4))

            # pair-0 Q/K on the fast (direct) drain path
            for s in range(2):
                for half in range(2):
                    qk_round(0, s, half, direct=True)

            rds = {}

            prod_sched = ([("qk", 1, s, half) for s in range(2) for half in range(2)] +
                          [("v", rc) for rc in range(8)] +
                          [("qk", j, s, half) for j in (2, 3)
                           for s in range(2) for half in range(2)])
            # 20 production steps; pair1 by i=2, V by i=6, pair2 by i=8
            prod_per_iter = [2, 2, 2, 2, 2, 2, 2, 2, 2, 2, 0, 0, 0, 0, 0, 0]

            def emit_prod(i):
                for _ in range(prod_per_iter[i]):
                    if prod_sched:
                        stp = prod_sched.pop(0)
                        if stp[0] == "v":
                            v_step(stp[1])
                        else:
                            qk_round(stp[1], stp[2], stp[3], direct=True)

            LAG = 6
            hist = {}

            def attn_front(i):
                j, hh, half = i // 4, (i // 2) % 2, i % 2
                hs = ds(half * 512, 512)
                pb = ds(64 * hh, 64)
                eA = ep.tile([P, 4, 512], F8, tag="e", name=f"eA{i}")
                eB = ep.tile([P, 2, 512], F8, tag="eb", name=f"eB{i}")
                eC = ep.tile([P, 2, 512], F8, tag="ec", name=f"eC{i}")
                sA = ps.tile([P, 2048], F32, tag="sA")
                for c in range(4):
                    nc.tensor.matmul(sA[:, ds(512 * c, 512)],
                                     KT[j][pb, :, ds(128 * c, 128)],
                                     QT[j][pb, :, hs], start=True, stop=True,
                                     perf_mode=PM)
                nc.scalar.activation(eA.rearrange("p a b -> p (a b)"), sA,
                                     AF.Exp, scale=EXP_SCALE)
                sB = ps.tile([P, 1024], F32, tag="sB", name=f"sB{i}")
                for c in range(2):
                    nc.tensor.matmul(sB[:, ds(512 * c, 512)],
                                     KT[j][pb, :, ds(128 * (4 + c), 128)],
                                     QT[j][pb, :, hs], start=True, stop=True,
                                     perf_mode=PM)
                nc.scalar.activation(eB.rearrange("p a b -> p (a b)"), sB,
                                     AF.Exp, scale=EXP_SCALE)
                if i >= LAG:
                    back1(i - LAG)
                sC = ps.tile([P, 1024], F32, tag="sB", name=f"sC{i}")
                for c in range(2):
                    nc.tensor.matmul(sC[:, ds(512 * c, 512)],
                                     KT[j][pb, :, ds(128 * (6 + c), 128)],
                                     QT[j][pb, :, hs], start=True, stop=True,
                                     perf_mode=PM)
                nc.scalar.activation(eC.rearrange("p a b -> p (a b)"), sC,
                                     AF.Exp, scale=EXP_SCALE)
                if i >= LAG + 1:
                    back2(i - LAG - 1)
                emit_prod(i)
                hist[i] = (eA, eB, eC)

            def back1(i):
                """denominator matmuls + reciprocal for iteration i."""
                eA, eB, eC = hist[i]
                dd = ps.tile([P, 512], F32, tag="pvdd", name=f"dd{i}")
                for cc in range(4):
                    e = (eA[:, 0:2], eA[:, 2:4], eB, eC)[cc]
                    nc.tensor.matmul(dd, ones8, e,
                                     start=(cc == 0), stop=(cc == 3), perf_mode=PM)
                rd = rdp.tile([P, 512], F32, tag="rd", name=f"rd{i}")
                nc.vector.reciprocal_approx_fast(rd, dd)
                rds[i] = rd

            def back2(i):
                """PV + normalized O drain for iteration i."""
                eA, eB, eC = hist.pop(i)
                rd = rds.pop(i)
                j, hh, half = i // 4, (i // 2) % 2, i % 2
                hs = ds(half * 512, 512)
                h = 2 * j + hh
                pv = ps.tile([P, 512], F32, tag="pvdd", name=f"pv{i}")
                for cc in range(4):
                    e = (eA[:, 0:2], eA[:, 2:4], eB, eC)[cc]
                    nc.tensor.matmul(pv, V[:, 2 * cc:2 * cc + 2, ds(128 * h, 128)],
                                     e, start=(cc == 0), stop=(cc == 3), perf_mode=PM)
                ob = stg.tile([P, 512], BF16, tag="ob", name=f"ob{i}")
                nc.vector.tensor_copy(ob, pv)
                if hh == 0:
                    nc.gpsimd.tensor_mul(O1T[0:64, j, hs], ob[0:64], rd[0:64])
                    nc.gpsimd.tensor_mul(O2T[64:128, j, hs], ob[64:128], rd[64:128])
                else:
                    nc.gpsimd.tensor_mul(O2T[0:64, j, hs], ob[0:64], rd[0:64])
                    nc.gpsimd.tensor_mul(O1T[64:128, j, hs], ob[64:128], rd[64:128])

            for i in range(16):
                attn_front(i)
            for i in range(16 - LAG, 16):
                back1(i)
                if i - 1 >= 16 - LAG - 1 and (i - 1) in rds:
                    back2(i - 1)
            back2(15)

            # ---- phase C ------------------------------------------------
            CT = sing.tile([P, 8, 2], F32)       # per-block [sum(z), sum(z^2)]
            MF = sing.tile([P, 8, 2], F32)       # per-block [mu, factor]
            zs = {}

            def proj_block(tb, zt, zoff):
                tsl = ds(tb * P, P)
                z = zt[:, ds(zoff, 1024)]
                nc.tensor.matmul(z[:, 0:512], ID, XN[:, tb, 0:512],
                                 start=True, stop=True)
                nc.tensor.matmul(z[:, 512:1024], ID, XN[:, tb, 512:1024],
                                 start=True, stop=True)
                for jj in range(2):
                    nc.tensor.matmul(z[:, 0:512], O1T[:, 2 * jj:2 * jj + 2, tsl],
                                     W1[:, 2 * jj:2 * jj + 2, 0:512],
                                     start=False, stop=(jj == 1), perf_mode=PM,
                                     skip_group_check=True)
                for jj in range(2):
                    nc.tensor.matmul(z[:, 512:768], O1T[:, 2 * jj:2 * jj + 2, tsl],
                                     W1[:, 2 * jj:2 * jj + 2, 512:768],
                                     start=False, stop=(jj == 1), perf_mode=PM,
                                     skip_group_check=True)
                for jj in range(2):
                    nc.tensor.matmul(z[:, 768:1024], O2T[:, 2 * jj:2 * jj + 2, tsl],
                                     W2[:, 2 * jj:2 * jj + 2, :],
                                     start=False, stop=(jj == 1), perf_mode=PM,
                                     skip_group_check=True)
                dmy = statp.tile([P, 1024], BF16, tag="dmy", name=f"dm{tb}")
                nc.scalar.activation(dmy, z, AF.Identity,
                                     accum_out=CT[:, tb, 0:1])
                nc.scalar.activation(dmy, z, AF.Square,
                                     accum_out=CT[:, tb, 1:2])
                zs[tb] = z

            def newton(t0, b=1):
                """1/(sigma+eps) for blocks [t0, t0+b) from CT -> MF."""
                sz = CT[:, ds(t0, b), 0]
                sq = CT[:, ds(t0, b), 1]
                mu = MF[:, ds(t0, b), 0]
                fc = MF[:, ds(t0, b), 1]
                w = statp.tile([P, b], F32, tag="w", name=f"w{t0}")
                y = statp.tile([P, b], F32, tag="y", name=f"y{t0}")
                t = statp.tile([P, b], F32, tag="t", name=f"t{t0}")
                nc.vector.tensor_scalar(mu, sz, 1.0 / N, None, ALU.mult)
                nc.vector.tensor_mul(w, sz, sz)
                nc.vector.tensor_scalar(w, w, -1.0 / N, None, ALU.mult)
                nc.vector.tensor_add(w, w, sq)      # w = (n-1) * unbiased var
                wi = w.bitcast(I32)
                yi = y.bitcast(I32)
                nc.vector.tensor_scalar(yi, wi, 1, None,
                                        ALU.logical_shift_right)
                nc.vector.tensor_scalar(yi, yi, -1, None, ALU.mult)
                nc.vector.tensor_scalar(yi, yi, MAGIC, None, ALU.add)
                for _ in range(2):                  # Newton: y = y*(1.5-0.5*w*y^2)
                    nc.vector.tensor_mul(t, w, y)
                    nc.vector.tensor_mul(t, t, y)
                    nc.vector.tensor_scalar(t, t, -0.5, 1.5, ALU.mult, ALU.add)
                    nc.vector.tensor_mul(y, y, t)
                # factor = sqrt(n-1)*y/(1+eps'*sqrt(n-1)*y) ~= y*(c0 - c1*y)
                c0 = float(np.sqrt(N - 1.0))
                c1 = float(RES * EPS * (N - 1.0))
                nc.vector.tensor_scalar(t, y, -c1, c0, ALU.mult, ALU.add)
                nc.vector.tensor_mul(fc, y, t)

            def norm_block(tb):
                tsl = ds(tb * P, P)
                ot = outp.tile([P, DM], F32, tag="o", name=f"ot{tb}")
                nc.vector.tensor_scalar(ot, zs[tb], MF[:, tb, 0:1],
                                        MF[:, tb, 1:2], ALU.subtract, ALU.mult)
                if apply_ln:
                    nc.vector.tensor_mul(ot, ot, LNA)
                    nc.vector.tensor_add(ot, ot, LNB)
                nc.sync.dma_start(out_d[tsl, :], ot)

            # z slots: sA holds two blocks (halves), sB one; per-block newton
            # so blocks pipeline across PE -> ACT -> DVE.
            slot_of = [0, 1, 2, 0, 1, 2, 0, 1]   # 0/1 = sA half, 2 = sB
            zA = None
            for tb in range(8):
                sl = slot_of[tb]
                if sl == 2:
                    zt = ps.tile([P, 1024], F32, tag="sB", name=f"zB{tb}")
                    proj_block(tb, zt, 0)
                else:
                    if sl == 0:
                        zA = ps.tile([P, 2048], F32, tag="sA", name=f"zA{tb}")
                    proj_block(tb, zA, 1024 * sl)
                newton(tb)
                norm_block(tb)

    nc.finalize()
    return nc


def _prep(inp, w_qs1, w_ks1, w_vs1, w_qs2, w_ks2, w_vs2, w_proj1, w_proj2):
    x = np.ascontiguousarray(np.asarray(inp, np.float32)).reshape(NCORES, L, DM)

    xts, xns = [], []
    for b in range(NCORES):
        xt = x[b].T.reshape(8, P, L).transpose(1, 0, 2)
        xts.append(np.ascontiguousarray(xt).astype(F8NP))
        xn = x[b].reshape(8, P, DM).transpose(1, 0, 2)
        xns.append(np.ascontiguousarray(xn).astype(BF16NP))

    wqk = np.empty((P, 4, 2, 8, P), np.float32)
    for j in range(4):
        for s, (wa, wb) in enumerate(((w_qs1, w_qs2), (w_ks1, w_ks2))):
            for f in range(6):
                for m_h, h in ((0, 2 * j), (1, 2 * j + 1)):
                    wqk[:, j, s, f, 64 * m_h:64 * m_h + 64] = \
                        wa[h, 128 * f:128 * (f + 1), :]
            for f in (6, 7):
                for m_h, h in ((0, 2 * j), (1, 2 * j + 1)):
                    wqk[:, j, s, f, 64 * m_h:64 * m_h + 64] = \
                        wb[h, 128 * (f - 6):128 * (f - 5), :]
    wqk = (wqk * WS).astype(F8NP)

    horder = [0, 2, 4, 6, 1, 3, 5, 7]
    wv = np.empty((P, 8, 512), np.float32)
    for f in range(6):
        for i, h in enumerate(horder):
            wv[:, f, 64 * i:64 * i + 64] = w_vs1[h, 128 * f:128 * (f + 1), :]
    for f in (6, 7):
        for i, h in enumerate(horder):
            wv[:, f, 64 * i:64 * i + 64] = w_vs2[h, 128 * (f - 6):128 * (f - 5), :]
    wv = (wv * WS).astype(F8NP)

    w1 = np.ascontiguousarray(
        (np.asarray(w_proj1, np.float32) * WS).reshape(4, P, 768)
        .transpose(1, 0, 2)).astype(F8NP)
    w2r = np.asarray(w_proj2, np.float32).reshape(4, 2, 64, 256)
    w2r = np.ascontiguousarray(w2r[:, ::-1].reshape(4, P, 256) * WS)
    w2 = np.ascontiguousarray(w2r.transpose(1, 0, 2)).astype(F8NP)

    ident = (np.eye(P, dtype=np.float32) * RES).astype(BF16NP)
    return x, xts, xns, wqk, wv, w1, w2, ident


_NC_CACHE = {}


def _get_nc(apply_ln):
    if apply_ln not in _NC_CACHE:
        _NC_CACHE[apply_ln] = build_nc(apply_ln)
    return _NC_CACHE[apply_ln]


def kernel(inp, w_qs1, w_ks1, w_vs1, w_qs2, w_ks2, w_vs2, w_proj1, w_proj2,
           ln_a, ln_b, batch_size, max_len, _trace=False):
    inp = np.asarray(inp, np.float32)
    assert int(batch_size) == NCORES and int(max_len) == L
    assert inp.shape == (NCORES * L, DM)

    ln_a = np.asarray(ln_a, np.float32).reshape(-1)
    ln_b = np.asarray(ln_b, np.float32).reshape(-1)
    apply_ln = not (np.all(ln_a == 1.0) and np.all(ln_b == 0.0))

    x, xts, xns, wqk, wv, w1, w2, ident = _prep(
        inp, np.asarray(w_qs1, np.float32), np.asarray(w_ks1, np.float32),
        np.asarray(w_vs1, np.float32), np.asarray(w_qs2, np.float32),
        np.asarray(w_ks2, np.float32), np.asarray(w_vs2, np.float32),
        np.asarray(w_proj1, np.float32), np.asarray(w_proj2, np.float32))

    nc = _get_nc(apply_ln)

    in_maps = []
    for b in range(NCORES):
        m = dict(xt=xts[b], xn=xns[b], wqk=wqk, wv=wv, w1=w1, w2=w2,
                 ident=ident)
        if apply_ln:
            m["lna"] = np.broadcast_to(ln_a, (P, DM)).copy()
            m["lnb"] = np.broadcast_to(ln_b, (P, DM)).copy()
        in_maps.append(m)

    res = run_bass_kernel_spmd(nc, in_maps, list(range(NCORES)), trace=_trace)
    out = np.concatenate([res.results[b]["out"] for b in range(NCORES)], 0)
    if _trace:
        return out, res
    return out
